# revision 8
# baseline (speedup 1.0000x reference)
"""CRF token-mean loss for Trainium2, data-parallel over 8 NeuronCores.

Denominator via a meet-in-the-middle forward/backward scan fused into ONE
chain over 53 partitions (rows 0-20: forward alpha, rows 32-52: backward
gamma, rows 21-31 zero padding for the 32-partition alignment rules):

    S_0   = X_0 * [exp(start); 0; exp(end)]
    S_k   = (W^T S_{k-1}) * X_k,  k = 1..511
    W     = blkdiag-ish: W[0:21,0:21] = Etil, W[32:53,32:53] = Etil^T,
            Etil = c * exp(trans), c = 2^-4.5
    Z*c^1023 = sum_i alpha_511[i] * (Etil gamma_512)[i]

X_k pairs x_k (fwd, lanes 0-20) with x_{1023-k} (bwd, lanes 32-52) in one
64-lane group; emissions are host-prepacked as bf16 [128, 512, 64] so each
PE transpose is exactly [128,128] and exp pages are uniform ([128,512] bf16,
8 pairs per page). The prescale c keeps the bf16 state in range so only 3
renorm events (k=128,256,384) are needed.

Numerator: one-hot tags (bf16 is_equal at DVE 2x mode, half-separated
layout) used for
  - emission score: fused multiply-accumulate with paired emissions
  - transition-pair counts: 6-step blocked gram matmuls accumulated in two
    PSUM tiles (fwd pairs in superdiag blocks, bwd pairs in subdiag blocks),
    DMA'd out raw; host does the count . transitions dot in f64.
"""

import numpy as np
import ml_dtypes

import concourse.bass as bass
import concourse.tile as tile
from concourse import bacc, mybir
from concourse.bass_utils import run_bass_kernel_spmd

F32 = mybir.dt.float32
BF16 = mybir.dt.bfloat16
U8 = mybir.dt.uint8

ALU = mybir.AluOpType
ACTF = mybir.ActivationFunctionType

N_CORES = 8
B, L, T = 1024, 1024, 21
BLOC = B // N_CORES          # 128 sequences per core
SW = 53                      # state width: fwd 0-20, pad 21-31, bwd 32-52
BOFF = 32                    # bwd lane offset inside a pair group
PW = 64                      # paired-emission group width
TRW = 2 * T                  # 42: paired-tags group width
MID = 511                    # scan steps per direction
J = L // 2                   # 512 paired columns
JC = 48                      # paired columns per DMA chunk
CHUNKS = [JC] * 10 + [J - JC * 10]      # 48*10 + 32
PAGE_J = 8                   # paired columns per x page ([128, 512])
N_PAGES = J // PAGE_J        # 64
RENORM_AT = (128, 256, 384)
LN2_40 = 40.0 * np.log(2.0)
C_LOG2 = -4.5                # prescale exponent: Etil = 2^C_LOG2 * exp(trans)

# blob byte offsets (per partition)
OFF_W = 0            # bf16 [53, 53] -> 106B
OFF_SE = 106         # bf16 [53, 1]
OFF_ONESC_BF = 108   # bf16 [53, 1] ones
OFF_ONESR_F = 112    # f32 [1, 53] ones -> 212B, ends 324
OFF_ONESC_F = 324    # f32 [53, 1] ones
OFF_STARTREP = 328   # f32 [128, 21] -> 84B, ends 412
OFF_ENDREP = 412     # f32 [128, 21] -> ends 496
OFF_IOTA = 496       # bf16 [128, 42]: 0..20, 0..20 -> 84B, ends 580
OFF_IDENT = 580      # bf16 [128, 128] -> 256B, ends 836
BLOB_BYTES = 1024

# outA column map (f32 [128, 320])
OA_EM = 0            # 0..21: per-chunk emission-score partials (fwd+bwd)
OA_SE = 22           # 22: start, 23: end partials
OA_MS = 24           # mask sum
OA_GF = 32           # 32..178: gram_f [126, 147] (extra block: bwd bounds)
OA_GB = 180          # 180..305: gram_b [126, 126]
OA_COLS = 320


def _build(nc):
    em_d = nc.dram_tensor("em", [BLOC, J * PW], BF16, kind="ExternalInput").ap()
    tr_d = nc.dram_tensor("tr", [BLOC, J * TRW], BF16,
                          kind="ExternalInput").ap()
    mask_d = nc.dram_tensor("mask", [BLOC, L], F32, kind="ExternalInput").ap()
    blob_d = nc.dram_tensor("blob", [128, BLOB_BYTES], U8,
                            kind="ExternalInput").ap()
    outa_d = nc.dram_tensor("outa", [BLOC, OA_COLS], F32,
                            kind="ExternalOutput").ap()
    outb_d = nc.dram_tensor("outb", [1, 512], F32, kind="ExternalOutput").ap()

    with tile.TileContext(nc) as tc:
        with (
            tc.tile_pool(name="singles", bufs=1) as singles,
            tc.tile_pool(name="embuf", bufs=2) as embuf,
            tc.tile_pool(name="trbuf", bufs=2) as trbuf,
            tc.tile_pool(name="mkbuf", bufs=2) as mkbuf,
            tc.tile_pool(name="scr", bufs=1) as scr,
            tc.tile_pool(name="xbuf", bufs=1) as xbuf,
            tc.tile_pool(name="state", bufs=1) as state,
            tc.tile_pool(name="small", bufs=2) as small,
            tc.tile_pool(name="ps_q", bufs=2, space="PSUM") as ps_q,
            tc.tile_pool(name="ps_x", bufs=2, space="PSUM") as ps_x,
            tc.tile_pool(name="ps_gf", bufs=1, space="PSUM") as ps_gf,
            tc.tile_pool(name="ps_gb", bufs=1, space="PSUM") as ps_gb,
            tc.tile_pool(name="ps_m", bufs=2, space="PSUM") as ps_m,
        ):
            blob = singles.tile([128, BLOB_BYTES], U8)
            nc.sync.dma_start(out=blob, in_=blob_d)

            def fview(off, n):
                return blob[:, off:off + 4 * n].bitcast(F32)

            def bview(off, n):
                return blob[:, off:off + 2 * n].bitcast(BF16)

            W = bview(OFF_W, SW)[0:SW, :]
            se = bview(OFF_SE, 1)[0:SW, :]
            onesc_bf = bview(OFF_ONESC_BF, 1)[0:SW, :]
            onesr_f = fview(OFF_ONESR_F, SW)[0:1, :]
            onesc_f = fview(OFF_ONESC_F, 1)[0:SW, :]
            startrep = fview(OFF_STARTREP, T)
            endrep = fview(OFF_ENDREP, T)
            iota = bview(OFF_IOTA, TRW)
            ident = bview(OFF_IDENT, 128)

            outa_sb = singles.tile([BLOC, OA_COLS], F32)
            outb_sb = singles.tile([1, 512], F32)
            nc.vector.memset(outa_sb, 0.0)

            # ---- mask sum ----
            mask_sb = singles.tile([BLOC, L], F32)
            nc.sync.dma_start(out=mask_sb, in_=mask_d)
            nc.vector.tensor_reduce(out=outa_sb[:, OA_MS:OA_MS + 1],
                                    in_=mask_sb, axis=mybir.AxisListType.XYZW,
                                    op=ALU.add)

            # ---- resident exp(em) pages: [128, 512] bf16, 8 pairs/page ----
            xpages = [xbuf.tile([128, 512], BF16, tag=f"xp{p}", name=f"xp{p}")
                      for p in range(N_PAGES)]

            def x_slice(k):
                p, r = divmod(k, PAGE_J)
                pb = (r % 2) * PW
                cb = (r // 2) * 128
                return xpages[p][pb:pb + SW, cb:cb + 128]

            gram_f = ps_gf.tile([126, 147], F32, name="gram_f")
            gram_b = ps_gb.tile([126, 126], F32, name="gram_b")
            gf_started = gb_started = False

            prev_mk = None
            prev_cnt = 0
            j0 = 0
            for ci, cnt in enumerate(CHUNKS):
                em_t = embuf.tile([BLOC, JC * PW], BF16, tag="em", name="em_t")
                nc.sync.dma_start(out=em_t[:, 0:cnt * PW],
                                  in_=em_d[:, j0 * PW:(j0 + cnt) * PW])
                tr_t = trbuf.tile([BLOC, JC * TRW], BF16, tag="tr",
                                  name="tr_t")
                nc.sync.dma_start(out=tr_t[:, 0:cnt * TRW],
                                  in_=tr_d[:, j0 * TRW:(j0 + cnt) * TRW])

                # one-hot tags (bf16, 2x mode), half-separated layout:
                # cols [0, cnt*21) = fwd, [cnt*21, 2*cnt*21) = bwd --
                # gram matmul RHS views must be single-free-dim
                mk = mkbuf.tile([BLOC, JC * TRW], BF16, tag="mk", name="mk")
                for half in (0, 1):
                    iota_v = bass.AP(
                        tensor=iota.tensor, offset=iota.offset + half * T,
                        ap=[iota.ap[0], [0, cnt], [1, T]])
                    tr_v = bass.AP(
                        tensor=tr_t.tensor, offset=tr_t.offset + half * T,
                        ap=[tr_t.ap[0], [TRW, cnt], [1, T]])
                    mk_o = bass.AP(
                        tensor=mk.tensor, offset=mk.offset + half * cnt * T,
                        ap=[mk.ap[0], [T, cnt], [1, T]])
                    nc.vector.tensor_tensor(out=mk_o, in0=tr_v, in1=iota_v,
                                            op=ALU.is_equal)

                # emission score partials (fwd half, bwd half)
                sc = scr.tile([BLOC, JC * TRW], BF16, tag="sc", name="sc")
                for half in (0, 1):
                    mk_v = bass.AP(
                        tensor=mk.tensor, offset=mk.offset + half * cnt * T,
                        ap=[mk.ap[0], [T, cnt], [1, T]])
                    em_v = bass.AP(
                        tensor=em_t.tensor,
                        offset=em_t.offset + half * BOFF,
                        ap=[em_t.ap[0], [PW, cnt], [1, T]])
                    sc_o = bass.AP(
                        tensor=sc.tensor, offset=sc.offset + half * cnt * T,
                        ap=[sc.ap[0], [T, cnt], [1, T]])
                    col = OA_EM + 2 * ci + half
                    nc.vector.scalar_tensor_tensor(
                        out=sc_o, in0=mk_v, scalar=1.0, in1=em_v,
                        op0=ALU.mult, op1=ALU.mult,
                        accum_out=outa_sb[:, col:col + 1])

                def mk_view(tile_, joff, half, nj, half_cnt=None):
                    hc = cnt if half_cnt is None else half_cnt
                    return bass.AP(
                        tensor=tile_.tensor,
                        offset=tile_.offset + (half * hc + joff) * T,
                        ap=[tile_.ap[0], [1, nj * T]])

                if ci == 0:
                    # start/end gathers: j=0 fwd col is l=0, bwd col is l=1023
                    sg = small.tile([BLOC, T], F32, tag="sg", name="sg")
                    nc.vector.scalar_tensor_tensor(
                        out=sg, in0=mk[:, 0:T], scalar=1.0, in1=startrep,
                        op0=ALU.mult, op1=ALU.mult,
                        accum_out=outa_sb[:, OA_SE:OA_SE + 1])
                    sg2 = small.tile([BLOC, T], F32, tag="sg", name="sg2")
                    nc.vector.scalar_tensor_tensor(
                        out=sg2, in0=mk[:, cnt * T:cnt * T + T], scalar=1.0,
                        in1=endrep, op0=ALU.mult, op1=ALU.mult,
                        accum_out=outa_sb[:, OA_SE + 1:OA_SE + 2])

                # ---- gram matmuls: fwd pairs / bwd pairs in this chunk ----
                last_chunk = ci == len(CHUNKS) - 1
                for half in (0, 1):
                    g = gram_f if half == 0 else gram_b
                    started = gf_started if half == 0 else gb_started
                    groups = []
                    jj = 0
                    while jj < cnt - 1:
                        nj = min(6, cnt - jj)
                        groups.append((jj, nj))
                        jj += nj - 1
                    for gi, (jj, nj) in enumerate(groups):
                        # gram_b is closed by its last group of the last
                        # chunk; gram_f by the middle-pair matmul below.
                        close = (last_chunk and half == 1
                                 and gi == len(groups) - 1)
                        v = mk_view(mk, jj, half, nj)
                        nc.tensor.matmul(
                            out=g[0:nj * T, 0:nj * T], lhsT=v, rhs=v,
                            start=not started, stop=close,
                            skip_group_check=True)
                        started = True
                    if half == 0:
                        gf_started = started
                    else:
                        gb_started = started

                if ci > 0:
                    # chunk-boundary pairs
                    nc.tensor.matmul(
                        out=gram_f[0:T, T:2 * T],
                        lhsT=mk_view(prev_mk, prev_cnt - 1, 0, 1,
                                     half_cnt=prev_cnt),
                        rhs=mk_view(mk, 0, 0, 1),
                        start=False, stop=False, skip_group_check=True)
                    nc.tensor.matmul(
                        out=gram_f[0:T, 126:147],
                        lhsT=mk_view(mk, 0, 1, 1),
                        rhs=mk_view(prev_mk, prev_cnt - 1, 1, 1,
                                    half_cnt=prev_cnt),
                        start=False, stop=False, skip_group_check=True)
                if last_chunk:
                    # middle pair (l=511 -> l=512); closes gram_f
                    nc.tensor.matmul(
                        out=gram_f[0:T, T:2 * T],
                        lhsT=mk_view(mk, cnt - 1, 0, 1),
                        rhs=mk_view(mk, cnt - 1, 1, 1),
                        start=False, stop=True, skip_group_check=True)
                prev_mk, prev_cnt = mk, cnt

                # ---- transposes ([128,128] each) + exp into pages ----
                for t in range(cnt // 2):
                    gj = j0 + 2 * t
                    src = bass.AP(tensor=em_t.tensor,
                                  offset=em_t.offset + 2 * t * PW,
                                  ap=[em_t.ap[0], [1, 128]])
                    p, r = divmod(gj, PAGE_J)
                    slot = r // 2
                    if slot == 0:
                        psx = ps_x.tile([128, 512], BF16, tag="psx",
                                        name="psx")
                    nc.tensor.transpose(
                        out=psx[:, slot * 128:(slot + 1) * 128],
                        in_=src, identity=ident)
                    if slot == 3:
                        nc.scalar.activation(out=xpages[p], in_=psx,
                                             func=ACTF.Exp)
                j0 += cnt

            # ================= scan =================
            S = state.tile([SW, 128], BF16)
            se_b = bass.AP(tensor=se.tensor, offset=se.offset,
                           ap=[se.ap[0], [0, 128]])
            nc.vector.tensor_tensor(out=S, in0=x_slice(0), in1=se_b,
                                    op=ALU.mult)
            ev = 0
            for k in range(1, MID + 1):
                q = ps_q.tile([SW, 128], F32, tag="q", name="q")
                nc.tensor.matmul(out=q, lhsT=W, rhs=S, start=True, stop=True)
                nc.vector.tensor_tensor(out=S, in0=q, in1=x_slice(k),
                                        op=ALU.mult)
                if k in RENORM_AT:
                    s_ps = ps_m.tile([1, 128], F32, tag="m", name="s_ps")
                    nc.tensor.matmul(out=s_ps, lhsT=onesc_bf, rhs=S,
                                     start=True, stop=True)
                    nc.scalar.activation(
                        out=outb_sb[:, ev * 128:(ev + 1) * 128], in_=s_ps,
                        func=ACTF.Ln)
                    ev += 1
                    r = small.tile([1, 128], F32, tag="r", name="r")
                    nc.vector.reciprocal(out=r, in_=s_ps)
                    rb = ps_m.tile([SW, 128], F32, tag="m", name="rb")
                    nc.tensor.matmul(out=rb, lhsT=onesr_f, rhs=r,
                                     start=True, stop=True)
                    nc.vector.tensor_tensor(out=S, in0=rb, in1=S, op=ALU.mult)

            # final: Z = sum_i alpha_511[i] * (Etil gamma_512)[i]
            qf = ps_q.tile([SW, 128], F32, tag="q", name="qf")
            nc.tensor.matmul(out=qf, lhsT=W, rhs=S, start=True, stop=True)
            tf = state.tile([T, 128], F32, name="tf")
            nc.vector.tensor_tensor(out=tf, in0=qf[BOFF:BOFF + T, :],
                                    in1=S[0:T, :], op=ALU.mult)
            zf = ps_m.tile([1, 128], F32, tag="m", name="zf")
            nc.tensor.matmul(out=zf, lhsT=onesc_f[0:T, :], rhs=tf,
                             start=True, stop=True)
            nc.scalar.activation(out=outb_sb[:, 384:512], in_=zf,
                                 func=ACTF.Ln, scale=2.0 ** 40)

            # drain grams and ship results
            nc.vector.tensor_copy(out=outa_sb[0:126, OA_GF:OA_GF + 147],
                                  in_=gram_f)
            nc.vector.tensor_copy(out=outa_sb[0:126, OA_GB:OA_GB + 126],
                                  in_=gram_b)
            nc.sync.dma_start(out=outa_d, in_=outa_sb)
            nc.sync.dma_start(out=outb_d, in_=outb_sb)

    return nc


_NC_CACHE = None


def _get_nc():
    global _NC_CACHE
    if _NC_CACHE is None:
        nc = bacc.Bacc("TRN2", target_bir_lowering=False, debug=False,
                       enable_asserts=False, num_devices=N_CORES)
        _build(nc)
        nc.compile()
        _NC_CACHE = nc
    return _NC_CACHE


def _make_blob(start, end, trans):
    BF = ml_dtypes.bfloat16
    c = 2.0 ** C_LOG2
    Etil = (c * np.exp(trans.astype(np.float64))).astype(BF)
    Wm = np.zeros((SW, SW), BF)
    Wm[0:T, 0:T] = Etil
    Wm[BOFF:BOFF + T, BOFF:BOFF + T] = Etil.T
    sev = np.zeros(SW, np.float64)
    sev[0:T] = np.exp(start.astype(np.float64))
    sev[BOFF:BOFF + T] = np.exp(end.astype(np.float64))
    sev = sev.astype(BF)

    blob = np.zeros((128, BLOB_BYTES), np.uint8)

    def put(off, arr2d):
        a = np.ascontiguousarray(arr2d)
        bb = a.view(np.uint8).reshape(a.shape[0], -1)
        blob[:bb.shape[0], off:off + bb.shape[1]] = bb

    put(OFF_W, Wm)
    put(OFF_SE, sev.reshape(SW, 1))
    put(OFF_ONESC_BF, np.ones((SW, 1), BF))
    put(OFF_ONESR_F, np.ones((1, SW), np.float32))
    put(OFF_ONESC_F, np.ones((SW, 1), np.float32))
    put(OFF_STARTREP, np.broadcast_to(start.astype(np.float32), (128, T)))
    put(OFF_ENDREP, np.broadcast_to(end.astype(np.float32), (128, T)))
    iota_r = np.concatenate([np.arange(T), np.arange(T)]).astype(BF)
    put(OFF_IOTA, np.broadcast_to(iota_r, (128, TRW)))
    put(OFF_IDENT, np.eye(128, dtype=BF))
    return blob


def kernel(emissions, tags, mask, start_transitions, end_transitions,
           transitions):
    BF = ml_dtypes.bfloat16
    em_bf = np.asarray(emissions, dtype=np.float32).astype(BF)     # [B, L, T]
    tg = np.asarray(tags).astype(BF)                               # [B, L]
    mk = np.asarray(mask).astype(np.float32)
    start = np.asarray(start_transitions, dtype=np.float32)
    end = np.asarray(end_transitions, dtype=np.float32)
    trans = np.asarray(transitions, dtype=np.float64)

    emp = np.zeros((B, J, PW), BF)
    emp[:, :, 0:T] = em_bf[:, 0:J]
    emp[:, :, BOFF:BOFF + T] = em_bf[:, ::-1][:, 0:J]
    trp = np.empty((B, J, TRW), BF)
    trp[:, :, 0:T] = tg[:, 0:J, None]
    trp[:, :, T:TRW] = tg[:, ::-1][:, 0:J, None]

    blob = _make_blob(start, end, trans)

    in_maps = []
    for ccc in range(N_CORES):
        sl = slice(ccc * BLOC, (ccc + 1) * BLOC)
        in_maps.append(dict(
            em=emp[sl].reshape(BLOC, J * PW),
            tr=trp[sl].reshape(BLOC, J * TRW),
            mask=mk[sl],
            blob=blob,
        ))

    nc = _get_nc()
    global _last_in_maps, _last_results
    _last_in_maps = in_maps
    res = run_bass_kernel_spmd(nc, in_maps, core_ids=list(range(N_CORES)))
    _last_results = res.results

    num = 0.0
    den = 0.0
    msum = 0.0
    lnc = C_LOG2 * np.log(2.0)
    for r in res.results:
        oa = r["outa"].astype(np.float64)
        ob = r["outb"].astype(np.float64).ravel()
        num += oa[:, OA_EM:OA_EM + 2 * len(CHUNKS)].sum()
        num += oa[:, OA_SE:OA_SE + 2].sum()
        msum += oa[:, OA_MS].sum()
        gf = oa[0:126, OA_GF:OA_GF + 147]
        gb = oa[0:126, OA_GB:OA_GB + 126]
        C = np.zeros((T, T))
        for a in range(5):
            C += gf[a * T:(a + 1) * T, (a + 1) * T:(a + 2) * T]
        C += gf[0:T, 126:147]
        for a in range(1, 6):
            C += gb[a * T:(a + 1) * T, (a - 1) * T:a * T]
        num += (C * trans).sum()
        lnb = ob[:384].reshape(3, 128)
        lnz = ob[384:]
        den += (lnz - LN2_40).sum() + 2.0 * lnb.sum() \
            - BLOC * (L - 1) * lnc
    return np.float32((num - den) / msum)
